# revision 6
# baseline (speedup 1.0000x reference)
"""Expert-parallel MoE kernel for Trainium2 (8 NeuronCores).

Sharding: core e owns expert e. The host computes the top-2 routing (in
float64) only to decide which token rows go to which core's shard; every
numerical value in the output is computed on device:
  - the gate (logits -> top-2 softmax weight for this core's expert) is
    recomputed on device from x and the replicated Wg/bg,
  - the expert MLP relu(x@W1+b1)@W2+b2 runs on device in float32r,
  - the per-token gate weight is applied on device.
The host gathers each expert's token rows (transposed, zero-padded to a
common capacity), runs the SPMD kernel, and scatter-adds the per-core
results into the full [T, D_OUT] output.
"""

import math
import os
import sys

import numpy as np

sys.path.insert(0, "/opt/trn_rl_repo")

P = 128
E = 8
DIN = 1024
DH = 4096
DO = 1024
KC = DIN // P   # 8  k-chunks of x / W1 contraction
HC = DH // P    # 32 h-chunks of W2 contraction
NCORES = 8
TBMAX = 512     # max tokens per block
BIG = 1.0e30

_compiled = {}
LAST_DISPATCH_S = None


def _build(blocks, reps):
    import concourse.mybir as mybir
    import concourse.tile as tile
    from concourse import bacc

    F32 = mybir.dt.float32
    F32R = mybir.dt.float32r

    cap = sum(blocks)
    S = cap // P
    NSBMAX = TBMAX // P

    nc = bacc.Bacc("TRN2", target_bir_lowering=False, debug=False,
                   num_devices=NCORES)

    xT = nc.dram_tensor("xT", [P, KC, cap], F32R, kind="ExternalInput").ap()
    W1m = nc.dram_tensor("W1m", [P, KC, DH], F32R, kind="ExternalInput").ap()
    W2m = nc.dram_tensor("W2m", [P, HC, DO], F32R, kind="ExternalInput").ap()
    Wgm = nc.dram_tensor("Wgm", [P, KC, E], F32R, kind="ExternalInput").ap()
    b1c = nc.dram_tensor("b1c", [P, HC], F32, kind="ExternalInput").ap()
    b2r = nc.dram_tensor("b2r", [P, DO], F32, kind="ExternalInput").ap()
    bgr = nc.dram_tensor("bgr", [P, E], F32, kind="ExternalInput").ap()
    sel4 = nc.dram_tensor("sel4", [P, NSBMAX, E], F32, kind="ExternalInput").ap()
    out = nc.dram_tensor("out", [S, P, DO], F32, kind="ExternalOutput").ap()

    with tile.TileContext(nc) as tc:
        with tc.tile_pool(name="const", bufs=1) as cpool, \
             tc.tile_pool(name="work", bufs=2) as wpool, \
             tc.tile_pool(name="wts1", bufs=2) as wpool1, \
             tc.tile_pool(name="wts2", bufs=3) as wpool2, \
             tc.tile_pool(name="gate", bufs=2) as gpool, \
             tc.tile_pool(name="ps", bufs=3, space="PSUM") as ps, \
             tc.tile_pool(name="ps2", bufs=1, space="PSUM") as psl2, \
             tc.tile_pool(name="psg", bufs=1, space="PSUM") as psg:

            wg_sb = cpool.tile([P, KC, E], F32R)
            nc.sync.dma_start(wg_sb[:], Wgm[:])
            bg_sb = cpool.tile([P, E], F32)
            nc.sync.dma_start(bg_sb[:], bgr[:])
            b1_sb = cpool.tile([P, HC], F32)
            nc.sync.dma_start(b1_sb[:], b1c[:])
            b2_sb = cpool.tile([P, DO], F32)
            nc.sync.dma_start(b2_sb[:], b2r[:])
            sel_sb = cpool.tile([P, NSBMAX, E], F32)
            nc.sync.dma_start(sel_sb[:], sel4[:])

            def body(_iv=None):
                s0 = 0
                for tb in blocks:
                    nsb = tb // P
                    t0 = s0 * P

                    xt = wpool.tile([P, KC, TBMAX], F32R, tag="xt", name="xt")[:, :, :tb]
                    nc.sync.dma_start(xt[:], xT[:, :, t0:t0 + tb])

                    # ---------- gate ----------
                    lgb = gpool.tile([P, NSBMAX, E], F32, tag="lgb", name="lgb")[:, :nsb]
                    for s in range(nsb):
                        gps = psg.tile([P, E], F32, tag="gps")
                        for kc in range(KC):
                            nc.tensor.matmul(
                                gps[:], xt[:, kc, s * P:(s + 1) * P],
                                wg_sb[:, kc, :],
                                start=(kc == 0), stop=(kc == KC - 1))
                        nc.vector.tensor_tensor(
                            lgb[:, s, :], gps[:], bg_sb[:],
                            mybir.AluOpType.add)

                    gw = gpool.tile([P, NSBMAX, 44], F32, tag="gw", name="gw")
                    _ncol = [0]

                    def g3(tag):
                        c = _ncol[0]; _ncol[0] += 1
                        return gw[:, :nsb, c:c + 1]

                    def g8(tag):
                        c = _ncol[0]; _ncol[0] += E
                        return gw[:, :nsb, c:c + E]

                    X = mybir.AxisListType.X
                    m1 = g3("m1")
                    nc.vector.reduce_max(m1[:], lgb[:], axis=X)
                    eq = g8("eq")
                    nc.vector.tensor_tensor(
                        eq[:], lgb[:], m1.to_broadcast([P, nsb, E]),
                        mybir.AluOpType.is_ge)
                    cnt = g3("cnt")
                    nc.vector.reduce_sum(cnt[:], eq[:], axis=X)
                    tmp = g8("tmp")
                    nc.vector.tensor_scalar_mul(tmp[:], eq[:], BIG)
                    nc.vector.tensor_sub(tmp[:], lgb[:], tmp[:])
                    m2 = g3("m2")
                    nc.vector.reduce_max(m2[:], tmp[:], axis=X)
                    # exact multi-way tie at the max: m2 = m1
                    msk = g3("msk")
                    nc.vector.tensor_scalar(
                        msk[:], cnt[:], 2.0, None, mybir.AluOpType.is_ge)
                    dd = g3("dd")
                    nc.vector.tensor_sub(dd[:], m1[:], m2[:])
                    nc.vector.tensor_tensor(dd[:], dd[:], msk[:],
                                            mybir.AluOpType.mult)
                    nc.vector.tensor_add(m2[:], m2[:], dd[:])
                    # w = exp(lsel - m1) / (1 + exp(m2 - m1))
                    lsel = g3("lsel")
                    wst = g8("wst")
                    nc.vector.tensor_tensor(wst[:], lgb[:], sel_sb[:, :nsb],
                                            mybir.AluOpType.mult)
                    nc.vector.reduce_sum(lsel[:], wst[:], axis=X)
                    d2 = g3("d2")
                    nc.vector.tensor_sub(d2[:], m2[:], m1[:])
                    e2 = g3("e2")
                    nc.scalar.activation(e2[:], d2[:],
                                         mybir.ActivationFunctionType.Exp)
                    den = g3("den")
                    nc.vector.tensor_scalar_add(den[:], e2[:], 1.0)
                    rec = g3("rec")
                    nc.vector.reciprocal(rec[:], den[:])
                    dsel = g3("dsel")
                    nc.vector.tensor_sub(dsel[:], lsel[:], m1[:])
                    wcol = gpool.tile([P, NSBMAX, 1], F32, tag="wcol", name="wcol")[:, :nsb]
                    nc.scalar.activation(wcol[:], dsel[:],
                                         mybir.ActivationFunctionType.Exp)
                    nc.vector.tensor_tensor(wcol[:], wcol[:], rec[:],
                                            mybir.AluOpType.mult)

                    # ---------- layer 1 ----------
                    hT = wpool.tile([P, HC, TBMAX], F32R, tag="hT", name="hT",
                                    bufs=1)[:, :, :tb]
                    for hcg in range(8):
                        w1t = wpool1.tile([P, KC, 4 * P], F32R, tag="w1t")
                        nc.sync.dma_start(
                            w1t[:], W1m[:, :, hcg * 4 * P:(hcg + 1) * 4 * P])
                        for j in range(4):
                            hc = hcg * 4 + j
                            ps1 = ps.tile([P, TBMAX], F32, tag="mm", name="mm")[:, :tb]
                            for kc in range(KC):
                                nc.tensor.matmul(
                                    ps1[:], w1t[:, kc, j * P:(j + 1) * P],
                                    xt[:, kc, :],
                                    start=(kc == 0), stop=(kc == KC - 1))
                            nc.scalar.activation(
                                hT[:, hc, :], ps1[:],
                                mybir.ActivationFunctionType.Relu,
                                bias=b1_sb[:, hc:hc + 1], scale=1.0)

                    # ---------- layer 2 ----------
                    ob = wpool.tile([P, NSBMAX, DO], F32, tag="ob", name="ob")[:, :nsb]
                    for dt in range(2):
                        ps2s = [
                            psl2.tile([P, TBMAX], F32, tag=f"l2_{s}", name=f"l2_{s}")[:, :TBMAX]
                            for s in range(nsb)
                        ]
                        for hcg2 in range(8):
                            w2t = wpool2.tile([P, 4, 512], F32R, tag="w2t")
                            nc.sync.dma_start(
                                w2t[:],
                                W2m[:, hcg2 * 4:(hcg2 + 1) * 4,
                                    dt * 512:(dt + 1) * 512])
                            for s in range(nsb):
                                for j in range(4):
                                    hc = hcg2 * 4 + j
                                    nc.tensor.matmul(
                                        ps2s[s][:, :512],
                                        hT[:, hc, s * P:(s + 1) * P],
                                        w2t[:, j, :],
                                        start=(hc == 0), stop=(hc == HC - 1))
                        for s in range(nsb):
                            nc.vector.tensor_add(
                                ob[:, s, dt * 512:(dt + 1) * 512],
                                ps2s[s][:, :512],
                                b2_sb[:, dt * 512:(dt + 1) * 512])
                    for s in range(nsb):
                        nc.vector.tensor_scalar_mul(
                            ob[:, s, :], ob[:, s, :], wcol[:, s, :])
                    nc.sync.dma_start(
                        out[s0:s0 + nsb].rearrange("s p d -> p s d"), ob[:])
                    s0 += nsb

            if reps > 1:
                with tc.For_i(0, reps, 1) as _i:
                    body(_i)
            else:
                body()

    nc.compile()
    return nc


def _get_compiled(blocks, reps):
    key = (tuple(blocks), reps)
    if key not in _compiled:
        _compiled[key] = _build(blocks, reps)
    return _compiled[key]


def kernel(x, Wg, bg, W1, b1, W2, b2):
    from concourse.bass_utils import run_bass_kernel_spmd

    x = np.ascontiguousarray(np.asarray(x, dtype=np.float32))
    Wg = np.ascontiguousarray(np.asarray(Wg, dtype=np.float32))
    bg = np.ascontiguousarray(np.asarray(bg, dtype=np.float32))
    W1 = np.ascontiguousarray(np.asarray(W1, dtype=np.float32))
    b1 = np.ascontiguousarray(np.asarray(b1, dtype=np.float32))
    W2 = np.ascontiguousarray(np.asarray(W2, dtype=np.float32))
    b2 = np.ascontiguousarray(np.asarray(b2, dtype=np.float32))

    T = x.shape[0]

    # Host-side routing (float64) decides the shards only.
    logits = x.astype(np.float64) @ Wg.astype(np.float64) + bg.astype(np.float64)
    top2 = np.argpartition(logits, -2, axis=1)[:, -2:]
    sel_mask = np.zeros((T, E), dtype=bool)
    sel_mask[np.arange(T)[:, None], top2] = True

    idx_e = [np.nonzero(sel_mask[:, e])[0] for e in range(E)]
    counts = [len(i) for i in idx_e]
    cap = max(P, int(math.ceil(max(counts) / P)) * P)
    nfull, rem = divmod(cap, TBMAX)
    blocks = [TBMAX] * nfull + ([rem] if rem else [])

    reps = int(os.environ.get("MOE_REPS", "1"))
    nc = _get_compiled(blocks, reps)

    NSBMAX = TBMAX // P
    Wgm = Wg.reshape(KC, P, E).transpose(1, 0, 2).copy()
    bgr = np.tile(bg, (P, 1))

    in_maps = []
    for e in range(E):
        n = counts[e]
        xe = np.zeros((cap, DIN), dtype=np.float32)
        xe[:n] = x[idx_e[e]]
        sel = np.zeros(E, dtype=np.float32)
        sel[e] = 1.0
        in_maps.append({
            "xT": np.ascontiguousarray(
                xe.T.reshape(KC, P, cap).transpose(1, 0, 2)),
            "W1m": np.ascontiguousarray(
                W1[e].reshape(KC, P, DH).transpose(1, 0, 2)),
            "W2m": np.ascontiguousarray(
                W2[e].reshape(HC, P, DO).transpose(1, 0, 2)),
            "Wgm": Wgm,
            "b1c": np.ascontiguousarray(b1[e].reshape(HC, P).T),
            "b2r": np.tile(b2[e], (P, 1)),
            "bgr": bgr,
            "sel4": np.tile(sel, (P, NSBMAX, 1)),
        })

    import time as _time
    _t0 = _time.time()
    res = run_bass_kernel_spmd(nc, in_maps, list(range(NCORES)))
    global LAST_DISPATCH_S
    LAST_DISPATCH_S = _time.time() - _t0

    outf = np.zeros((T, DO), dtype=np.float32)
    for e in range(E):
        oe = res.results[e]["out"].reshape(cap, DO)
        outf[idx_e[e]] += oe[:counts[e]]
    return outf
